# revision 21
# baseline (speedup 1.0000x reference)
"""Trainium2 Bass kernel for nn_NMPN (GNN message passing), 8 NeuronCores.

Algorithm (reference):
    h0 = relu(fatoms @ W_nin.T)                       [50000, 512]
    H = h0
    repeat 4x:
        msg_h = concat([zeros(1,512), H[in_n]])        [120000, 512]
        msg   = concat([msg_h, fbonds], 1)             [120000, 523]
        nei   = msg[aoutgraph].sum(1)                  [50000, 523]
        H     = relu(h0 + nei @ W_node.T)              [50000, 512]
    return H.T

v2 design:
  - atoms row-sharded over 8 cores (6250 each); per core the atoms are
    re-ordered: [even-global-id atoms sorted by (#even neighbors) desc,
    pad to 3200] ++ [odd-id atoms likewise, pad to 3200] -> 6400 slots,
    50 chunks of 128.
  - H exchange tables are split BY GLOBAL-ID PARITY: T_even / T_odd
    [25601, 512] (row 25600 = zeros), fp8(e4m3) by default, Shared
    (per-HBM-pair) scratch, double-buffered A/B across depths. Each
    parity's 25 chunks are AllGathered in 4 segment groups (7/6/6/6)
    so collectives overlap compute.
  - gathers use gpsimd.dma_gather (int16 idx, one instr per
    (target-group, parity): ~16 instrs/depth instead of 294 INDIRECT1D)
    with per-chunk block counts B0/B1 = max neighbor count in that
    parity over the chunk's atoms (host-sorted so chunks are uniform).
  - per chunk: PE copy-accumulates the gathered blocks into PSUM via an
    identity-stationary matmul (nei, atom-major, fp32 sums), transposes
    via 4 PE matmuls, then the 512x512 main matmul in fp32r; relu+cast
    to fp8 and store; the fbonds term is folded into a depth-invariant
    `base` as in v1 (base = relu(h0) + fbg @ W_node[:,512:].T).
"""

import os
import numpy as np
import ml_dtypes

import concourse.bass as bass
import concourse.mybir as mybir
import concourse.tile as tile
from concourse import bacc
from concourse.bass_utils import run_bass_kernel_spmd
from concourse.masks import make_identity
from concourse.library_config import mlp as MLP_LIB

NCORES = 8
N_ATOMS = 50000
N_BONDS = 120000
MAX_NB = 6
ATOM_FDIM = 39
BOND_FDIM = 11
HIDDEN = 512
DEPTH = 4

A_LOC = N_ATOMS // NCORES        # 6250 atoms per core
P_CNT = A_LOC // 2               # 3125 atoms of each parity per core
P_LOC = 3200                     # padded slots per parity per core
NCH_P = P_LOC // 128             # 25 chunks per parity
NCHUNK = 2 * NCH_P               # 50 chunks per core
SLOTS = 2 * P_LOC                # 6400 agin rows per core
T_ROWS = NCORES * P_LOC          # 25600 data rows per parity table
ZROW = T_ROWS                    # zero row index within each table
FB = MAX_NB * BOND_FDIM          # 66

# AllGather segment groups per parity (chunk ranges within the parity)
_PG = os.environ.get("TRN_PAR_GROUPS", "0,9,17,25")
_PGB = [int(x) for x in _PG.split(",")]
PAR_GROUPS = list(zip(_PGB[:-1], _PGB[1:]))
NGRP = len(PAR_GROUPS)          # segment groups per parity
NGRP2 = 2 * NGRP                # total target groups (both parities)

F32 = mybir.dt.float32
F32R = mybir.dt.float32r
BF16 = mybir.dt.bfloat16
I16 = mybir.dt.int16

TDT_NAME = os.environ.get("TRN_TDT", "fp8")
TDT = mybir.dt.float8e4 if TDT_NAME == "fp8" else BF16
TDT_NP = mybir.dt.np(TDT)
SHARED = bool(int(os.environ.get("TRN_SHARED_TABLES", "1")))


def _group_of_chunk(cp):
    for g, (c0, c1) in enumerate(PAR_GROUPS):
        if c0 <= cp < c1:
            return g
    raise AssertionError(cp)


def _table_row(core, sp):
    """Row in the parity table for (core, slot-within-parity sp)."""
    cp = sp // 128
    g = _group_of_chunk(cp)
    c0, c1 = PAR_GROUPS[g]
    n = (c1 - c0) * 128
    return NCORES * c0 * 128 + core * n + (sp - c0 * 128)


def build_nc(B0, B1, NBE, NBO, idx_cols_e, idx_cols_o):
    """B0/B1: per-chunk (50) block counts; NBE/NBO: per target-group (8)
    total blocks; idx_cols_*: per target-group column counts of the two
    resident idx tensors."""
    nc = bacc.Bacc("TRN2", target_bir_lowering=False, num_devices=NCORES,
                   num_swdge_queues=int(os.environ.get("TRN_NSWQ", "4")))

    # ---- per-core external I/O ----
    fatoms_t = nc.dram_tensor("fatoms_t", [ATOM_FDIM, SLOTS], F32R, kind="ExternalInput")
    fbg_t = nc.dram_tensor("fbg_t", [FB, SLOTS], F32R, kind="ExternalInput")
    idx_e = nc.dram_tensor("idx_e", [128, sum(idx_cols_e)], I16, kind="ExternalInput")
    idx_o = nc.dram_tensor("idx_o", [128, sum(idx_cols_o)], I16, kind="ExternalInput")
    w_nin_t = nc.dram_tensor("w_nin_t", [ATOM_FDIM, HIDDEN], F32R, kind="ExternalInput")
    wb_rep = nc.dram_tensor("wb_rep", [FB, HIDDEN], F32R, kind="ExternalInput")
    w_h_t = nc.dram_tensor("w_h_t", [HIDDEN, HIDDEN], F32R, kind="ExternalInput")
    h_out = nc.dram_tensor("h_out", [SLOTS, HIDDEN], F32, kind="ExternalOutput")

    agin = nc.dram_tensor("agin", [SLOTS, HIDDEN], TDT)
    addr_space = "Shared" if SHARED else "Local"
    tables = [
        [
            nc.dram_tensor(f"table_{p}_{b}", [T_ROWS + 1, HIDDEN], TDT,
                           addr_space=addr_space)
            for b in range(2)
        ]
        for p in range(2)
    ]
    seed_dram = nc.dram_tensor("seed_dram", [1, 16], F32)

    rg = [list(range(NCORES))]
    NSWQ = nc.num_swdge_queues
    gsems = [nc.alloc_semaphore(f"gsem{q}") for q in range(NSWQ)]
    gcount = [0] * NSWQ
    gcall = [0]

    ecol0 = np.cumsum([0] + idx_cols_e).tolist()
    ocol0 = np.cumsum([0] + idx_cols_o).tolist()
    # per-chunk block offsets within its group tile
    cumB0, cumB1 = {}, {}
    for g in range(NGRP2):
        c0, c1 = PAR_GROUPS[g % NGRP]
        cbase = (g // NGRP) * NCH_P
        off = 0
        for c in range(c0, c1):
            cumB0[cbase + c] = off
            off += B0[cbase + c]
        off = 0
        for c in range(c0, c1):
            cumB1[cbase + c] = off
            off += B1[cbase + c]

    dbg_nocc = bool(int(os.environ.get("TRN_DBG_NOCC", "0")))

    def maybe_seg_ag(c, buf):
        """If chunk c (global, 0..49) ends a parity segment group, AllGather
        that group's agin rows into the parity table buffer `buf`."""
        par, cp = c // NCH_P, c % NCH_P
        for g, (c0, c1) in enumerate(PAR_GROUPS):
            if cp == c1 - 1:
                a0 = par * P_LOC + c0 * 128
                n_at = (c1 - c0) * 128
                row0 = NCORES * c0 * 128
                if dbg_nocc:
                    return nc.sync.dma_start(
                        out=tables[par][buf][row0:row0 + n_at, :],
                        in_=agin[a0:a0 + n_at, :],
                    )
                return nc.gpsimd.collective_compute(
                    "AllGather", mybir.AluOpType.bypass,
                    replica_groups=rg,
                    ins=[agin[a0:a0 + n_at, :]],
                    outs=[tables[par][buf][row0:row0 + NCORES * n_at, :]],
                )
        return None

    with tile.TileContext(nc) as tc:
        nc.gpsimd.load_library(MLP_LIB)
        with (
            tc.tile_pool(name="persist", bufs=1) as pp,
            tc.tile_pool(name="psum", bufs=2, space="PSUM") as psp,
            tc.tile_pool(name="gather", bufs=(2 if TDT_NAME == "fp8" else 1)) as gp,
            tc.tile_pool(name="work", bufs=3) as wp,
            tc.tile_pool(name="out", bufs=2) as op,
        ):
            # ---------- resident tiles ----------
            base_t = pp.tile([128, NCHUNK * HIDDEN], BF16, tag="base")
            ident_q = pp.tile([128, 128], TDT, tag="identq")
            make_identity(nc, ident_q[:, :])
            ident_b = pp.tile([128, 128], BF16, tag="identb")
            make_identity(nc, ident_b[:, :])
            idxe_sb = pp.tile([128, sum(idx_cols_e)], I16, tag="idxe")
            idxo_sb = pp.tile([128, sum(idx_cols_o)], I16, tag="idxo")
            l1 = nc.sync.dma_start(out=idxe_sb[:, :], in_=idx_e[:, :])
            l2 = nc.sync.dma_start(out=idxo_sb[:, :], in_=idx_o[:, :])
            whs = pp.tile([128, 4 * HIDDEN], F32R, tag="wh")
            for b in range(4):
                nc.sync.dma_start(
                    out=whs[:, b * HIDDEN:(b + 1) * HIDDEN],
                    in_=w_h_t[b * 128:(b + 1) * 128, :],
                )
            zeros_q = pp.tile([1, HIDDEN], TDT, tag="zr")
            nc.vector.memset(zeros_q[:, :], 0.0)
            zeros_f = pp.tile([1, 16], F32, tag="zf")
            nc.vector.memset(zeros_f[:, :], 0.0)
            nc.sync.dma_start(out=seed_dram[:, :], in_=zeros_f[:, :])
            zrow_w = [
                nc.sync.dma_start(out=tables[p][b][ZROW:ZROW + 1, :],
                                  in_=zeros_q[:, :])
                for p in range(2) for b in range(2)
            ]
            nc.vector.memset(base_t[:, :], 0.0)

            # ---------- setup: base = relu(fatoms@Wnin.T) + fbg@Wbrep ----------
            ccs = {"e": None, "o": None}
            with tc.tile_pool(name="setup", bufs=3) as sp:
                wnin_sb = sp.tile([ATOM_FDIM, HIDDEN], F32R, tag="wnin")
                nc.sync.dma_start(out=wnin_sb[:, :], in_=w_nin_t[:, :])
                wbr_sb = sp.tile([FB, HIDDEN], F32R, tag="wbr")
                nc.sync.dma_start(out=wbr_sb[:, :], in_=wb_rep[:, :])

                for c in range(NCHUNK):
                    a0 = c * 128
                    fa_sb = sp.tile([ATOM_FDIM, 128], F32R, tag="fa")
                    nc.sync.dma_start(out=fa_sb[:, :], in_=fatoms_t[:, a0:a0 + 128])
                    fbg_sb = sp.tile([FB, 128], F32R, tag="fbg")
                    nc.sync.dma_start(out=fbg_sb[:, :], in_=fbg_t[:, a0:a0 + 128])
                    ps_h0 = psp.tile([128, HIDDEN], F32, tag="ps_n")
                    nc.tensor.matmul(
                        out=ps_h0[:, :], lhsT=fa_sb[:, :], rhs=wnin_sb[:, :],
                        start=True, stop=True,
                    )
                    ps_b = psp.tile([128, HIDDEN], F32, tag="ps_t")
                    nc.tensor.matmul(
                        out=ps_b[:, :], lhsT=fbg_sb[:, :], rhs=wbr_sb[:, :],
                        start=True, stop=True,
                    )
                    h0f = op.tile([128, HIDDEN], F32, tag="h0f")
                    nc.scalar.activation(
                        h0f[:, :], ps_h0[:, :], mybir.ActivationFunctionType.Relu,
                    )
                    nc.vector.tensor_add(
                        base_t[:, c * HIDDEN:(c + 1) * HIDDEN], h0f[:, :], ps_b[:, :],
                    )
                    h0q = op.tile([128, HIDDEN], TDT, tag="h0q")
                    nc.vector.tensor_copy(h0q[:, :], h0f[:, :])
                    nc.sync.dma_start(out=agin[a0:a0 + 128, :], in_=h0q[:, :])
                    ccx = maybe_seg_ag(c, 0)
                    if ccx is not None:
                        ccs["e" if c < NCH_P else "o"] = ccx

            # seed the Pool sequencer clock (gathers are 1-wait DMA ops).
            def seed(dep_insts, tag):
                prev = None
                for i, d in enumerate(dep_insts):
                    if d is None:
                        continue
                    st = wp.tile([1, 16], F32, tag=f"seed_{tag}_{i}")
                    s = nc.gpsimd.dma_start(out=st[:, :], in_=seed_dram[:, :])
                    tile.add_dep_helper(s.ins, d.ins, sync=True, reason=f"seed {tag}")
                    if prev is not None:
                        tile.add_dep_helper(s.ins, prev.ins, sync=False, reason="chain")
                    prev = s
                return prev

            # ---------- depth loop ----------
            dbg_depth = int(os.environ.get("TRN_DBG_DEPTH", str(DEPTH)))
            dbg_nogather = bool(int(os.environ.get("TRN_DBG_NOGATHER", "0")))
            dbg_nowait = bool(int(os.environ.get("TRN_DBG_NOWAIT", "0")))
            MAXB = int(os.environ.get("TRN_MAXB", "8"))

            def emit_part(info, tbl, idx_sb, col0, nblk, gt_tile):
                b0 = 0
                while b0 < nblk:
                    bn = min(MAXB, nblk - b0)
                    qn = gcall[0] % NSWQ
                    gcall[0] += 1
                    gi = nc.gpsimd.dma_gather(
                        out_ap=gt_tile[:, b0:b0 + bn, :],
                        in_ap=tbl[:, :],
                        idxs_ap=idx_sb[:, col0 + b0 * 8:col0 + (b0 + bn) * 8],
                        num_idxs=bn * 128,
                        num_idxs_reg=bn * 128,
                        elem_size=HIDDEN,
                        queue_num=qn,
                    )
                    gi.then_inc(gsems[qn], 16)
                    gcount[qn] += 1
                    info["wq"][qn] = gcount[qn]
                    info["gis"].append((gi, qn))
                    b0 += bn

            for d in range(dbg_depth):
                t_e = tables[0][d % 2]
                t_o = tables[1][d % 2]
                last = d == dbg_depth - 1
                ginfo = {}

                def start_group(g):
                    """Allocate group tiles + emit its even-source gathers."""
                    gte = gp.tile([128, NBE[g], HIDDEN], TDT, tag="gte")
                    gto = gp.tile([128, NBO[g], HIDDEN], TDT, tag="gto")
                    ginfo[g] = dict(gte=gte, gto=gto, wq={}, gis=[])
                    if dbg_nogather:
                        nc.vector.memset(gte[:, :, :], 0.0)
                        nc.vector.memset(gto[:, :, :], 0.0)
                        return
                    emit_part(ginfo[g], t_e, idxe_sb, ecol0[g], NBE[g], gte)

                def finish_group(g):
                    """Emit the group's odd-source gathers."""
                    if dbg_nogather:
                        return
                    emit_part(ginfo[g], t_o, idxo_sb, ocol0[g], NBO[g],
                              ginfo[g]["gto"])

                # group-0 even-source gathers only need the even tables,
                # which finished AllGathering halfway through the previous
                # depth -- so they overlap the previous depth's odd half
                # and the odd-parity AllGathers.
                deps_e = [ccs["e"]] + ([l1, l2] + zrow_w if d == 0 else [])
                seed(deps_e, f"d{d}e")
                start_group(0)
                seed([ccs["o"]], f"d{d}o")
                finish_group(0)

                for g in range(NGRP2):
                    c0p, c1p = PAR_GROUPS[g % NGRP]
                    cbase = (g // NGRP) * NCH_P
                    if g + 1 < NGRP2:
                        start_group(g + 1)
                        finish_group(g + 1)
                    info = ginfo.pop(g)
                    gte, gto = info["gte"], info["gto"]
                    ws = []
                    if info["gis"] and not dbg_nowait:
                        for q, cnt in sorted(info["wq"].items()):
                            w = nc.tensor.wait_ge(gsems[q], 16 * cnt)
                            ws.append(w)
                            for gi, gq in info["gis"]:
                                if gq == q:
                                    tile.add_dep_helper(w.ins, gi.ins, sync=False,
                                                        reason="w after g")

                    for cp in range(c0p, c1p):
                        c = cbase + cp
                        a0 = c * 128
                        blocks = (
                            [(gte, cumB0[c] + i) for i in range(B0[c])]
                            + [(gto, cumB1[c] + i) for i in range(B1[c])]
                        )
                        assert blocks
                        # nei (atom-major) = sum of gathered blocks, via
                        # identity-stationary copy-accumulate matmuls
                        ps_n = psp.tile([128, HIDDEN], F32, tag="ps_n")
                        for i, (gt_t, blk) in enumerate(blocks):
                            mm = nc.tensor.matmul(
                                out=ps_n[:, :],
                                lhsT=ident_q[:, :],
                                rhs=gt_t[:, blk, :],
                                start=(i == 0), stop=(i == len(blocks) - 1),
                            )
                            if i == 0:
                                for w in ws:
                                    tile.add_dep_helper(
                                        mm.ins, w.ins, sync=False, reason="mm after wait")
                        ntA = wp.tile([128, HIDDEN], BF16, tag="ntA")
                        nc.scalar.copy(ntA[:, :], ps_n[:, :])
                        # transpose: ps_t[p, b*128+a] = nei[a, b*128+p]
                        ps_t = psp.tile([128, HIDDEN], F32, tag="ps_t")
                        for b in range(4):
                            nc.tensor.matmul(
                                out=ps_t[:, b * 128:(b + 1) * 128],
                                lhsT=ntA[:, b * 128:(b + 1) * 128],
                                rhs=ident_b[:, :],
                                start=True, stop=True,
                            )
                        nt = wp.tile([128, HIDDEN], F32R, tag="nt")
                        nc.vector.tensor_copy(nt[:, :], ps_t[:, :])
                        # H_new = relu(base + neiT.T @ W_h.T)
                        ps_o = psp.tile([128, HIDDEN], F32, tag="ps_o")
                        for b in range(4):
                            nc.tensor.matmul(
                                out=ps_o[:, :],
                                lhsT=nt[:, b * 128:(b + 1) * 128],
                                rhs=whs[:, b * HIDDEN:(b + 1) * HIDDEN],
                                start=(b == 0), stop=(b == 3),
                            )
                        tnew = op.tile([128, HIDDEN], F32, tag="tnew")
                        nc.vector.tensor_add(
                            tnew[:, :], ps_o[:, :],
                            base_t[:, c * HIDDEN:(c + 1) * HIDDEN],
                        )
                        if last:
                            hf = op.tile([128, HIDDEN], F32, tag="hf")
                            nc.scalar.activation(
                                hf[:, :], tnew[:, :],
                                mybir.ActivationFunctionType.Relu,
                            )
                            nc.sync.dma_start(out=h_out[a0:a0 + 128, :], in_=hf[:, :])
                        else:
                            hq = op.tile([128, HIDDEN], TDT, tag="hq")
                            nc.scalar.activation(
                                hq[:, :], tnew[:, :],
                                mybir.ActivationFunctionType.Relu,
                            )
                            nc.sync.dma_start(out=agin[a0:a0 + 128, :], in_=hq[:, :])
                            ccx = maybe_seg_ag(c, (d + 1) % 2)
                            if ccx is not None:
                                ccs["e" if c < NCH_P else "o"] = ccx

    nc.finalize()
    return nc


def _prepare(fatoms, fbonds, W_nin, W_node, aoutgraph, in_n):
    fatoms = np.asarray(fatoms, dtype=np.float32)
    fbonds = np.asarray(fbonds, dtype=np.float32)
    W_nin = np.asarray(W_nin, dtype=np.float32)
    W_node = np.asarray(W_node, dtype=np.float32)
    aout = np.asarray(aoutgraph, dtype=np.int64)
    in_n = np.asarray(in_n, dtype=np.int64)

    # source atom (global id) per (atom, nb-slot); -1 -> zero message
    src = np.where(aout > 0, in_n[np.maximum(aout - 1, 0)], -1)  # [50000, 6]
    parity = (np.arange(N_ATOMS) % 2).astype(np.int64)
    src_par = np.where(src >= 0, src % 2, -1)
    n_e = (src_par == 0).sum(1)  # per-atom even-neighbor count

    # --- per-core slot assignment (sorted by n_e desc within parity) ---
    slot = np.full(N_ATOMS, -1, dtype=np.int64)        # atom -> local slot
    slot_atom = np.full((NCORES, SLOTS), -1, np.int64)  # (core, slot) -> atom
    for k in range(NCORES):
        ids = np.arange(k * A_LOC, (k + 1) * A_LOC)
        for p in range(2):
            grp = ids[ids % 2 == p]
            order = grp[np.argsort(-n_e[grp], kind="stable")]
            s0 = p * P_LOC
            slot[order] = s0 + np.arange(len(order))
            slot_atom[k, s0:s0 + len(order)] = order

    # table row (within its parity table) of each atom
    trow = np.full(N_ATOMS, -1, np.int64)
    for k in range(NCORES):
        for p in range(2):
            for sp in range(P_CNT):
                a = slot_atom[k, p * P_LOC + sp]
                trow[a] = _table_row(k, sp)

    # --- per-chunk block counts (max across cores; SPMD-uniform) ---
    B0 = np.zeros(NCHUNK, np.int64)
    B1 = np.zeros(NCHUNK, np.int64)
    for k in range(NCORES):
        for c in range(NCHUNK):
            atoms = slot_atom[k, c * 128:(c + 1) * 128]
            atoms = atoms[atoms >= 0]
            if len(atoms) == 0:
                continue
            ne = n_e[atoms]
            no = (src_par[atoms] == 1).sum(1)
            B0[c] = max(B0[c], ne.max())
            B1[c] = max(B1[c], no.max())
    B0 = np.maximum(B0, 1).tolist()
    B1 = np.maximum(B1, 1).tolist()

    NBE = []
    NBO = []
    for g in range(NGRP2):
        c0, c1 = PAR_GROUPS[g % NGRP]
        cbase = (g // NGRP) * NCH_P
        NBE.append(int(sum(B0[cbase + c] for c in range(c0, c1))))
        NBO.append(int(sum(B1[cbase + c] for c in range(c0, c1))))
    idx_cols_e = [nb * 8 for nb in NBE]   # (nb*128)/16 columns
    idx_cols_o = [nb * 8 for nb in NBO]

    # --- weights ---
    w_nin_t = np.ascontiguousarray(W_nin.T)                      # [39, 512]
    w_h_t = np.ascontiguousarray(W_node[:, :HIDDEN].T)           # [512, 512]
    wb = W_node[:, HIDDEN:]                                      # [512, 11]
    wb_rep = np.ascontiguousarray(np.tile(wb.T, (MAX_NB, 1)))    # [66, 512]

    def wrap16(flat):
        # flat int16 index list -> [128, len/16] wrapped + replicated
        n = len(flat)
        assert n % 16 == 0
        return np.tile(flat.reshape(n // 16, 16).T, (8, 1)).astype(np.int16)

    in_maps = []
    for k in range(NCORES):
        fa = np.zeros((ATOM_FDIM, SLOTS), np.float32)
        fbg = np.zeros((FB, SLOTS), np.float32)
        idxe_parts, idxo_parts = [], []
        for g in range(NGRP2):
            c0, c1 = PAR_GROUPS[g % NGRP]
            cbase = (g // NGRP) * NCH_P
            fe = np.full((NBE[g] * 128,), ZROW, np.int16)
            fo = np.full((NBO[g] * 128,), ZROW, np.int16)
            be_off = 0
            bo_off = 0
            for c in range(c0, c1):
                cg = cbase + c
                for pos in range(128):
                    a = slot_atom[k, cg * 128 + pos]
                    if a < 0:
                        continue
                    nbs = src[a]
                    ev = [trow[s] for s in nbs[(nbs >= 0) & (nbs % 2 == 0)]]
                    od = [trow[s] for s in nbs[(nbs >= 0) & (nbs % 2 == 1)]]
                    assert len(ev) <= B0[cg] and len(od) <= B1[cg], (len(ev), len(od))
                    for i, r in enumerate(ev):
                        fe[(be_off + i) * 128 + pos] = r
                    for i, r in enumerate(od):
                        fo[(bo_off + i) * 128 + pos] = r
                be_off += B0[cg]
                bo_off += B1[cg]
            assert be_off == NBE[g] and bo_off == NBO[g]
            idxe_parts.append(wrap16(fe))
            idxo_parts.append(wrap16(fo))

        # per-slot features
        for s in range(SLOTS):
            a = slot_atom[k, s]
            if a < 0:
                continue
            fa[:, s] = fatoms[a]
            fbg[:, s] = fbonds[aout[a]].reshape(FB)

        in_maps.append({
            "fatoms_t": fa,
            "fbg_t": fbg,
            "idx_e": np.concatenate(idxe_parts, axis=1),
            "idx_o": np.concatenate(idxo_parts, axis=1),
            "w_nin_t": w_nin_t,
            "wb_rep": wb_rep,
            "w_h_t": w_h_t,
        })

    meta = dict(B0=B0, B1=B1, NBE=NBE, NBO=NBO,
                idx_cols_e=idx_cols_e, idx_cols_o=idx_cols_o,
                slot=slot, slot_atom=slot_atom)
    return in_maps, meta


_cache = {}


def run(inputs, trace=False):
    in_maps, meta = _prepare(**inputs)
    key = (tuple(meta["B0"]), tuple(meta["B1"]))
    if key not in _cache:
        _cache[key] = build_nc(meta["B0"], meta["B1"], meta["NBE"], meta["NBO"],
                               meta["idx_cols_e"], meta["idx_cols_o"])
    nc = _cache[key]
    res = run_bass_kernel_spmd(
        nc, in_maps, core_ids=list(range(NCORES)), trace=trace
    )
    H = np.empty((N_ATOMS, HIDDEN), np.float32)
    for k in range(NCORES):
        hk = res.results[k]["h_out"]          # [6400, 512]
        ids = np.arange(k * A_LOC, (k + 1) * A_LOC)
        H[ids] = hk[meta["slot"][ids]]
    out = np.ascontiguousarray(H.T)
    return out, res


def kernel(**inputs) -> np.ndarray:
    trace = bool(int(os.environ.get("TRN_KERNEL_TRACE", "0")))
    out, _ = run(inputs, trace=trace)
    return out

